# revision 18
# baseline (speedup 1.0000x reference)
"""Exp-kernel multivariate Hawkes process log-likelihood on 8 Trainium2 cores.

Data-parallel: one sequence (L=2048) per core. The O(L^2) pairwise sum is
split into (a) a within-block band (16 blocks of 128) computed densely via
a K=20 bf16 one-hot matmul + exp, and (b) the strictly-older "prefix" part
computed in O(L*D^2) via per-type-pair sufficient statistics (Ogata-style):

  prefix_i = sum_{(a,d)} X3[(a,d), i] * G[(a,d), k(i)]
  X3[(a,d), i] = [e_i = a] * alpha*beta[a,d] * exp(-b[a,d](t_i - c_k))
  G[(a,d), k]  = sum_{j < 128k, e_j = d} exp(-b[a,d](c_k - t_j))

H (per-block partial sums) and G (cross-block decayed scan) are carried on
the vector engine with a single tensor_tensor_scan; the per-event combine is
16 tiny PE matmuls contracting the 100 type-pairs. Arguments for the two big
exp() calls are built on the tensor engine from one shared (16, 2048) bf16
rhs with compensated hi/lo rows; masking of wrong-type entries uses -2048
offsets that underflow exp to exactly 0. The strict-triangle band mask is
accumulated in PSUM by 4 extra matmuls (lower-tri ones stationary times
-2048*I moving) so the scalar engine exps masked PSUM directly. Big exp
outputs are bf16, which doubles vector-engine reduction throughput. Total
scalar-engine exp work is ~3x2048 elements/lane (vs ~17400 for the dense
O(L^2) triangle).

Final ln(lam) runs on-device (exp+ln share one ACT table set via a scoped
patch of the table catalog passed to the insertion pass), and the result is
collapsed to a (1, 17) row with a ones-matmul so the output DMA is a single
packet (a (128, 1) output costs ~7us in straggling per-engine completions).

Engine programs and semaphores are hand-written (raw bacc, no Tile context).
"""
import numpy as np

B, L, D = 8, 2048, 10
NB = L // 128            # 16 blocks of 128 events
P2 = D * D               # 100 type pairs (a, d)
MASKC = 2048.0           # exp(-2048 + small) == 0, and 2048 >> |args| keeps
                         # fp32 accumulation exact to ~2^-12
NCORES = 8

# rp (shared rhs + stationaries), bf16, (128, 768). Strip c (partitions
# 32c:32c+16) carries rhs chunk c plus its own stationary copies so the four
# arg matmuls run concurrently in separate PE row groups:
#   cols 0:512     rhs rows over j-chunk c: [one, E1 d=0..9, one, one, tch,
#                  tch, tcl]
#   cols 512:640   SX stationary (X3-args), col 100:128 -> -MASKC row0 only
#   cols 640:768   SH stationary (H-args)
RHS_W = 512
SX_OFF = 512
SH_OFF = 640
RP_W = 768

# cA f32 (128, 193): [mug 0:16 | argc 16:176 | ones 176 | Delta 177:193]
MUG_OFF = 0
ARGC_OFF = 16
ONES_OFF = 176
DELTA_OFF = 177
CA_W = 193

# mk bf16 (128, 784): [LT ones (m>=c) 0:128 | 4x -2048*I 128:640 |
#                      I128 640:768 | mug 768:784]
MK_W = 784

_CACHE = {}


def _build_nc():
    import concourse.bacc as bacc
    from concourse import mybir
    from contextlib import ExitStack, contextmanager

    f32 = mybir.dt.float32
    bf16 = mybir.dt.bfloat16
    Alu = mybir.AluOpType
    Act = mybir.ActivationFunctionType

    @contextmanager
    def _one_act_table():
        """Scope-patch the activation-table catalog fed to bass's table-load
        insertion pass so Exp and Ln resolve to the single combined
        natural_log_exp_and_others set (true set indices are preserved; only
        the per-set function membership seen by the chooser shrinks)."""
        orig = bacc.get_activation_tables
        exp_t = mybir.ActivationFunctionType.Exp
        ln_t = mybir.ActivationFunctionType.Ln

        def patched(arch):
            tables = dict(orig(arch))
            if "natural_log_exp_and_others" not in tables:
                return tables
            out = {}
            for name, funcs in tables.items():
                if name != "natural_log_exp_and_others":
                    funcs = funcs - {exp_t, ln_t}
                out[name] = funcs
            return out

        bacc.get_activation_tables = patched
        try:
            yield
        finally:
            bacc.get_activation_tables = orig

    nc = bacc.Bacc()
    RP = nc.declare_dram_parameter("rp", [128, RP_W], bf16, isOutput=False)
    CA = nc.declare_dram_parameter("ca", [128, CA_W], f32, isOutput=False)
    UV = nc.declare_dram_parameter("uv", [20, 2 * L], bf16, isOutput=False)
    MK = nc.declare_dram_parameter("mk", [128, MK_W], bf16, isOutput=False)
    OUT = nc.declare_dram_parameter("out", [1, NB + 1], f32, isOutput=True)

    with ExitStack() as ctx:
        ctx.enter_context(nc.allow_low_precision(
            "bf16 block-sum outputs; 0.4% relative on positive sums is far "
            "inside the 2e-2 gate"))
        rp = ctx.enter_context(nc.sbuf_tensor([128, RP_W], bf16))
        ca = ctx.enter_context(nc.sbuf_tensor([128, CA_W], f32))
        uv = ctx.enter_context(nc.sbuf_tensor([20, 2 * L], bf16))
        mk = ctx.enter_context(nc.sbuf_tensor([128, MK_W], bf16))
        x3 = ctx.enter_context(nc.sbuf_tensor([128, L], bf16))
        hexp = ctx.enter_context(nc.sbuf_tensor([100, L], bf16))
        expb = ctx.enter_context(nc.sbuf_tensor([128, L], bf16))
        hsb = ctx.enter_context(nc.sbuf_tensor([100, NB], bf16))
        tmp = ctx.enter_context(nc.sbuf_tensor([100, NB], f32))
        gb = ctx.enter_context(nc.sbuf_tensor([128, NB], bf16))
        exc = ctx.enter_context(nc.sbuf_tensor([128, D * NB], f32))
        pdiag = ctx.enter_context(nc.sbuf_tensor([128, NB], bf16))
        lamA = ctx.enter_context(nc.sbuf_tensor([128, NB], f32))
        lamB = ctx.enter_context(nc.sbuf_tensor([128, NB], f32))
        acc = ctx.enter_context(nc.sbuf_tensor([128, NB + 1], f32))
        osb = ctx.enter_context(nc.sbuf_tensor([1, NB + 1], f32))
        warmt = ctx.enter_context(nc.sbuf_tensor([128, 512], bf16))
        psA = ctx.enter_context(nc.psum_tensor([128, 2048], f32))
        psB = ctx.enter_context(nc.psum_tensor([128, 2048], f32))
        warm_sem = ctx.enter_context(nc.semaphore("warm_sem"))
        rp_sem = ctx.enter_context(nc.semaphore("rp_sem"))
        ca_sem = ctx.enter_context(nc.semaphore("ca_sem"))
        uv_sem = ctx.enter_context(nc.semaphore("uv_sem"))
        mk_sem = ctx.enter_context(nc.semaphore("mk_sem"))
        pe_sem = ctx.enter_context(nc.semaphore("pe_sem"))
        act_sem = ctx.enter_context(nc.semaphore("act_sem"))
        dve_sem = ctx.enter_context(nc.semaphore("dve_sem"))
        gp_sem = ctx.enter_context(nc.semaphore("gp_sem"))
        out_sem = ctx.enter_context(nc.semaphore("out_sem"))
        block = ctx.enter_context(nc.Block(no_gpsimd_drain=True))
        rp, ca, uv, mk, x3, hexp, expb = (
            rp[:], ca[:], uv[:], mk[:], x3[:], hexp[:], expb[:])
        warmt = warmt[:]
        hsb, tmp, gb, exc, pdiag, lamA, lamB, acc, osb = (
            hsb[:], tmp[:], gb[:], exc[:], pdiag[:], lamA[:], lamB[:],
            acc[:], osb[:])
        psA, psB = psA[:], psB[:]

        mug = ca[:, MUG_OFF:MUG_OFF + NB]
        argc = ca[:, ARGC_OFF:ARGC_OFF + D * NB]
        onescol = ca[:, ONES_OFF:ONES_OFF + 1]
        delta = ca[0:100, DELTA_OFF:DELTA_OFF + NB]

        @block.sync
        def _(sync):
            sync.dma_start(out=rp, in_=RP[:]).then_inc(rp_sem, 16)
            sync.dma_start(out=ca, in_=CA[:]).then_inc(ca_sem, 16)
            sync.wait_ge(dve_sem, 10)
            sync.dma_start(out=OUT[:], in_=osb, single_packet=True).then_inc(out_sem, 16)

        @block.gpsimd
        def _(gp):
            gp.dma_start(out=uv, in_=UV[:]).then_inc(uv_sem, 16)
            gp.dma_start(out=mk, in_=MK[:]).then_inc(mk_sem, 16)
            nc.gpsimd.memset(tmp[:, 0:1], 0.0).then_inc(gp_sem, 1)
            # zero gb rows 96:128 (base must be 32-aligned); the scan later
            # overwrites rows 96:100 with real G values
            nc.gpsimd.memset(gb[96:128, :], 0.0).then_inc(gp_sem, 1)

        @block.tensor
        def _(pe):
            # HAM warm-up: big dummy matmuls while input DMAs land so the
            # real matmuls run at 2.4 GHz instead of the cold 1.2 GHz
            pe.wait_ge(warm_sem, 1)
            for _w in range(4):
                nc.tensor.matmul(
                    psB[:, 1024:1536], warmt[:, 0:128], warmt,
                    start=True, stop=True,
                )
            pe.wait_ge(rp_sem, 16)
            # H-args -> psA; four concurrent row-group matmuls
            for c in range(4):
                nc.tensor.matmul(
                    psA[:, 512 * c:512 * (c + 1)],
                    rp[32 * c:32 * c + 16, SH_OFF:SH_OFF + 128],
                    rp[32 * c:32 * c + 16, 0:512],
                    start=True, stop=True, tile_position=(32 * c, 0),
                ).then_inc(pe_sem, 1)                      # pe 1-4
            # X3-args -> psB
            for c in range(4):
                nc.tensor.matmul(
                    psB[:, 512 * c:512 * (c + 1)],
                    rp[32 * c:32 * c + 16, SX_OFF:SX_OFF + 128],
                    rp[32 * c:32 * c + 16, 0:512],
                    start=True, stop=True, tile_position=(32 * c, 0),
                ).then_inc(pe_sem, 1)                      # pe 5-8
            # band: -2048 strict-lower-triangle mask base, then W accumulates
            pe.wait_ge(mk_sem, 16)
            pe.wait_ge(uv_sem, 16)
            pe.wait_ge(act_sem, 1)   # psA half 1 free (H-exp chunk 1 read)
            for c in range(2):
                nc.tensor.matmul(
                    psA[:, 512 * c:512 * (c + 1)],
                    mk[:, 0:128],
                    mk[:, 128:640],
                    start=True, stop=False, skip_group_check=True,
                ).then_inc(pe_sem, 1)                      # pe 9-10
            pe.wait_ge(act_sem, 1)   # psA free
            for c in range(2, 4):
                nc.tensor.matmul(
                    psA[:, 512 * c:512 * (c + 1)],
                    mk[:, 0:128],
                    mk[:, 128:640],
                    start=True, stop=False, skip_group_check=True,
                ).then_inc(pe_sem, 1)                      # pe 11-12
            for k in range(NB):
                nc.tensor.matmul(
                    psA[:, 128 * k:128 * (k + 1)],
                    uv[:, 128 * k:128 * (k + 1)],
                    uv[:, L + 128 * k:L + 128 * (k + 1)],
                    start=False, stop=(k % 4 == 3), skip_group_check=True,
                ).then_inc(pe_sem, 1)                      # pe 13-28
            # P[p,k] = sum_{(a,d)} X3[(a,d), 128k+p] * G[(a,d), k]
            pe.wait_ge(act_sem, 2)   # psB free (X3-exp read), X3 ready
            pe.wait_ge(dve_sem, 3)   # G scan done
            pe.wait_ge(gp_sem, 2)    # gb rows 96:128 zeroed
            # open the P accumulation group with the mug base: psB[:,0:16] = I @ mug
            nc.tensor.matmul(
                psB[:, 0:NB],
                mk[:, 640:768],
                mk[:, 768:768 + NB],
                start=True, stop=False, skip_group_check=True,
            ).then_inc(pe_sem, 1)                          # pe 29
            for k in range(NB):
                nc.tensor.matmul(
                    psB[:, k:k + 1],
                    x3[:, 128 * k:128 * (k + 1)],
                    gb[:, k:k + 1],
                    start=False, stop=(k == NB - 1), skip_group_check=True,
                ).then_inc(pe_sem, 1)                      # pe 30-45
            # partition-sum of acc via ones-matmul -> (1, 17)
            pe.wait_ge(act_sem, 8)
            pe.wait_ge(dve_sem, 9)
            nc.tensor.matmul(
                psB[0:1, 512:512 + NB + 1],
                onescol,
                acc,
                start=True, stop=True,
            ).then_inc(pe_sem, 1)                          # pe 46

        @block.scalar
        def _(act):
            act.wait_ge(pe_sem, 4)
            nc.scalar.activation(out=hexp, in_=psA[0:100, :], func=Act.Exp
                                 ).then_inc(act_sem, 1)    # act 1
            act.wait_ge(pe_sem, 8)
            nc.scalar.activation(out=x3, in_=psB, func=Act.Exp
                                 ).then_inc(act_sem, 1)    # act 2
            for c in range(4):
                act.wait_ge(pe_sem, 16 + 4 * c)  # band quarter c masked+summed
                nc.scalar.activation(
                    out=expb[:, 512 * c:512 * (c + 1)],
                    in_=psA[:, 512 * c:512 * (c + 1)],
                    func=Act.Exp).then_inc(act_sem, 1)     # act 3-6
            act.wait_ge(ca_sem, 16)
            nc.scalar.activation(out=exc, in_=argc, func=Act.Exp
                                 ).then_inc(act_sem, 1)    # act 7
            act.wait_ge(dve_sem, 8)
            nc.scalar.activation(out=acc[:, 0:NB], in_=lamB, func=Act.Ln
                                 ).then_inc(act_sem, 1)    # act 8

        @block.vector
        def _(dve):
            nc.vector.memset(warmt, 0.0).then_inc(warm_sem, 1)
            # H block sums (100, 16)
            dve.wait_ge(act_sem, 1)
            nc.vector.tensor_reduce(
                out=hsb, in_=hexp.rearrange("p (k x) -> p k x", k=NB),
                axis=mybir.AxisListType.X, op=Alu.add,
            ).then_inc(dve_sem, 1)                         # dve 1
            # tmp[:,k] = Delta[:,k] * H[:,k-1]
            dve.wait_ge(ca_sem, 16)
            dve.wait_ge(gp_sem, 2)   # tmp col0 + gb tail memsets done
            dve.wait_ge(dve_sem, 1)  # same-engine W->R on hsb
            nc.vector.tensor_tensor(
                out=tmp[:, 1:NB], in0=delta[:, 1:NB], in1=hsb[0:100, 0:NB - 1],
                op=Alu.mult,
            ).then_inc(dve_sem, 1)                         # dve 2
            # G[:,k] = Delta[:,k]*G[:,k-1] + tmp[:,k]   (bf16 out)
            dve.wait_ge(dve_sem, 2)  # same-engine W->R on tmp
            nc.vector.tensor_tensor_scan(
                out=gb[0:100, :], data0=delta, data1=tmp[0:100, :],
                initial=0.0, op0=Alu.mult, op1=Alu.add,
            ).then_inc(dve_sem, 1)                         # dve 3
            # band row sums, four quarters pipelined with band-exp
            for c in range(4):
                dve.wait_ge(act_sem, 3 + c)
                nc.vector.tensor_reduce(
                    out=pdiag[:, 4 * c:4 * (c + 1)],
                    in_=expb[:, 512 * c:512 * (c + 1)].rearrange(
                        "p (k x) -> p k x", k=4),
                    axis=mybir.AxisListType.X, op=Alu.add,
                ).then_inc(dve_sem, 1)                     # dve 4-7
            # lam = (P + mug) + pdiag
            dve.wait_ge(pe_sem, 45)
            dve.wait_ge(dve_sem, 7)  # same-engine W->R on pdiag
            nc.vector.tensor_tensor(
                out=lamB, in0=psB[:, 0:NB], in1=pdiag, op=Alu.add,
            ).then_inc(dve_sem, 1)                         # dve 8
            # compensator row sums (parallel with Ln)
            dve.wait_ge(act_sem, 7)
            nc.vector.tensor_reduce(
                out=acc[:, NB:NB + 1], in_=exc,
                axis=mybir.AxisListType.X, op=Alu.add,
            ).then_inc(dve_sem, 1)                         # dve 9
            # copy (1,17) result out of psum
            dve.wait_ge(pe_sem, 46)
            nc.vector.tensor_scalar_add(
                osb, psB[0:1, 512:512 + NB + 1], 0.0,
            ).then_inc(dve_sem, 1)                         # dve 10

    with _one_act_table():
        nc.finalize()
    return nc


def _softplus(x):
    return np.logaddexp(0.0, np.asarray(x, np.float64))


def _bf(x):
    import ml_dtypes
    return np.asarray(x, np.float32).astype(ml_dtypes.bfloat16)


def _host_prep(time_points, T, mu_raw, log_alpha, log_beta, event_types):
    """Per-core input tiles + additive host constants."""
    import ml_dtypes
    bf16 = ml_dtypes.bfloat16

    mu = _softplus(mu_raw).astype(np.float32)
    alpha = _softplus(log_alpha).astype(np.float32)
    beta = _softplus(log_beta).astype(np.float32)
    lnab = np.log(alpha.astype(np.float64) * beta.astype(np.float64)).astype(np.float32)
    lna = np.log(alpha.astype(np.float64)).astype(np.float32)
    colsumA = alpha.sum(0, dtype=np.float64)
    mu_sum = mu.sum(dtype=np.float64)

    b_flat = beta.reshape(-1).astype(np.float64)                  # b[a,d]
    A_flat = (alpha.astype(np.float64) * beta.astype(np.float64)).reshape(-1)
    lnA_flat = np.log(A_flat)
    bh = _bf(b_flat); bl = _bf(b_flat - bh.astype(np.float64))
    lAh = _bf(lnA_flat); lAl = _bf(lnA_flat - lAh.astype(np.float64))

    # stationaries (16, 128) f32 (bf16-representable values by construction)
    sel_d = np.zeros((D, P2), np.float32)
    sel_a = np.zeros((D, P2), np.float32)
    for a in range(D):
        for d in range(D):
            sel_d[d, D * a + d] = MASKC
            sel_a[a, D * a + d] = MASKC
    SX = np.zeros((16, 128), np.float32)
    SX[0, :] = -MASKC
    SX[1:11, :P2] = sel_a
    SX[11, :P2] = lAh.astype(np.float32)
    SX[12, :P2] = lAl.astype(np.float32)
    SX[13, :P2] = -bh.astype(np.float32)
    SX[14, :P2] = -bl.astype(np.float32)
    SX[15, :P2] = -bh.astype(np.float32)
    SH = np.zeros((16, 128), np.float32)
    SH[0, :] = -MASKC
    SH[1:11, :P2] = sel_d
    SH[13, :P2] = bh.astype(np.float32)
    SH[14, :P2] = bl.astype(np.float32)
    SH[15, :P2] = bh.astype(np.float32)

    # mk: lower-tri ones (m >= c) + 4 copies of -2048*I
    lt = (np.arange(128)[:, None] >= np.arange(128)[None, :]).astype(np.float32)
    negi = np.zeros((128, 512), np.float32)
    for r in range(4):
        negi[:, 128 * r:128 * (r + 1)] = -MASKC * np.eye(128, dtype=np.float32)
    mk_const = np.concatenate(
        [lt, negi, np.eye(128, dtype=np.float32)], axis=1)        # (128, 768)

    in_maps, consts = [], []
    for bb in range(B):
        t = np.asarray(time_points[bb], np.float32)
        e = np.asarray(event_types[bb], np.int64)
        Tb = np.float64(T[bb])

        c = t.reshape(NB, 128).mean(axis=1).astype(np.float32)
        tc = (t - np.repeat(c, 128)).astype(np.float32)
        tch = _bf(tc)
        tcl = _bf(tc - tch.astype(np.float32))
        E1 = np.zeros((L, D), np.float32)
        E1[np.arange(L), e] = 1.0

        one = np.ones(L, np.float32)
        rhs = np.stack(
            [one] + [E1[:, d] for d in range(D)]
            + [one, one, tch.astype(np.float32), tch.astype(np.float32),
               tcl.astype(np.float32)], axis=0)             # (16, L)
        rp = np.zeros((128, RP_W), np.float32)
        for cc in range(4):
            rp[32 * cc:32 * cc + 16, 0:512] = rhs[:, 512 * cc:512 * (cc + 1)]
            rp[32 * cc:32 * cc + 16, SX_OFF:SX_OFF + 128] = SX
            rp[32 * cc:32 * cc + 16, SH_OFF:SH_OFF + 128] = SH

        # Delta for the scan; col 0 = 0 resets the recursion
        Delta = np.zeros((100, NB), np.float32)
        dcs = np.diff(c.astype(np.float64))
        for k in range(1, NB):
            Delta[:, k] = np.exp(-b_flat * dcs[k - 1]).astype(np.float32)

        mug = mu[e].reshape(NB, 128).T.copy()               # (128, NB)
        dt = (np.float32(Tb) - t).astype(np.float32)
        argc = (lna[:, e] - beta[:, e] * dt[None, :]).astype(np.float32)
        argc = argc.reshape(D, NB, 128).transpose(2, 0, 1).reshape(128, D * NB)

        ca = np.zeros((128, CA_W), np.float32)
        ca[:, MUG_OFF:MUG_OFF + NB] = mug
        ca[:, ARGC_OFF:ARGC_OFF + D * NB] = argc
        ca[:, ONES_OFF] = 1.0
        ca[0:100, DELTA_OFF:DELTA_OFF + NB] = Delta

        # band stacks: W = U @ V^T per 128-block, times recentered at c_k
        U = np.empty((L, 2 * D), np.float32)
        U[:, :D] = lnab[e, :] - beta[e, :] * tc[:, None]
        U[:, D:] = beta[e, :]
        V = np.concatenate([E1, E1 * tc[:, None]], axis=1)
        uvt = np.concatenate([_bf(U).T, _bf(V).T], axis=1)  # (20, 2L)

        mkt = np.concatenate(
            [mk_const, mug.astype(np.float32)], axis=1).astype(bf16)
        const = -Tb * mu_sum - colsumA[e].sum()
        in_maps.append({
            "rp": np.ascontiguousarray(rp.astype(bf16)),
            "ca": np.ascontiguousarray(ca),
            "uv": np.ascontiguousarray(uvt.astype(bf16)),
            "mk": np.ascontiguousarray(mkt),
        })
        consts.append(const)
    return in_maps, consts


def kernel(**inputs):
    from concourse.bass_utils import run_bass_kernel_spmd

    if "nc" not in _CACHE:
        _CACHE["nc"] = _build_nc()
    nc = _CACHE["nc"]

    in_maps, consts = _host_prep(**inputs)
    res = run_bass_kernel_spmd(nc, in_maps, list(range(NCORES)))
    out = np.empty(B, np.float32)
    for bb in range(B):
        row = res.results[bb]["out"].reshape(NB + 1).astype(np.float64)
        out[bb] = np.float32(row.sum() + consts[bb])
    return out


# revision 19
# speedup vs baseline: 1.0385x; 1.0385x over previous
"""Exp-kernel multivariate Hawkes process log-likelihood on 8 Trainium2 cores.

Data-parallel: one sequence (L=2048) per core. The O(L^2) pairwise sum is
split into (a) a within-block band (16 blocks of 128) computed densely via
a K=20 bf16 one-hot matmul + exp, and (b) the strictly-older "prefix" part
computed in O(L*D^2) via per-type-pair sufficient statistics (Ogata-style):

  prefix_i = sum_{(a,d)} X3[(a,d), i] * G[(a,d), k(i)]
  X3[(a,d), i] = [e_i = a] * alpha*beta[a,d] * exp(-b[a,d](t_i - c_k))
  G[(a,d), k]  = sum_{j < 128k, e_j = d} exp(-b[a,d](c_k - t_j))

H (per-block partial sums) and G (cross-block decayed scan) are carried on
the vector engine with a single tensor_tensor_scan; the per-event combine is
16 tiny PE matmuls contracting the 100 type-pairs. Arguments for the two big
exp() calls are built on the tensor engine from one shared (16, 2048) bf16
rhs with compensated hi/lo rows; masking of wrong-type entries uses -2048
offsets that underflow exp to exactly 0. The strict-triangle band mask is
accumulated in PSUM by 4 extra matmuls (lower-tri ones stationary times
-2048*I moving) so the scalar engine exps masked PSUM directly. Big exp
outputs are bf16, which doubles vector-engine reduction throughput. Total
scalar-engine exp work is ~3x2048 elements/lane (vs ~17400 for the dense
O(L^2) triangle).

Final ln(lam) runs on-device (exp+ln share one ACT table set via a scoped
patch of the table catalog passed to the insertion pass), and the result is
collapsed to a (1, 17) row with a ones-matmul so the output DMA is a single
packet (a (128, 1) output costs ~7us in straggling per-engine completions).

Engine programs and semaphores are hand-written (raw bacc, no Tile context).
"""
import numpy as np

B, L, D = 8, 2048, 10
NB = L // 128            # 16 blocks of 128 events
P2 = D * D               # 100 type pairs (a, d)
MASKC = 2048.0           # exp(-2048 + small) == 0, and 2048 >> |args| keeps
                         # fp32 accumulation exact to ~2^-12
NCORES = 8

# rp (shared rhs + stationaries), bf16, (128, 768). Strip c (partitions
# 32c:32c+16) carries rhs chunk c plus its own stationary copies so the four
# arg matmuls run concurrently in separate PE row groups:
#   cols 0:512     rhs rows over j-chunk c: [one, E1 d=0..9, one, one, tch,
#                  tch, tcl]
#   cols 512:640   SX stationary (X3-args), col 100:128 -> -MASKC row0 only
#   cols 640:768   SH stationary (H-args)
RHS_W = 512
SX_OFF = 512
SH_OFF = 640
RP_W = 768

# cA f32 (128, 193): [mug 0:16 | argc 16:176 | ones 176 | Delta 177:193]
MUG_OFF = 0
ARGC_OFF = 16
ONES_OFF = 176
DELTA_OFF = 177
CA_W = 193

# mk bf16 (128, 784): [LT ones (m>=c) 0:128 | 4x -2048*I 128:640 |
#                      I128 640:768 | mug 768:784]
MK_W = 784

_CACHE = {}


def _build_nc():
    import concourse.bacc as bacc
    from concourse import mybir
    from contextlib import ExitStack, contextmanager

    f32 = mybir.dt.float32
    bf16 = mybir.dt.bfloat16
    Alu = mybir.AluOpType
    Act = mybir.ActivationFunctionType

    @contextmanager
    def _one_act_table():
        """Scope-patch the activation-table catalog fed to bass's table-load
        insertion pass so Exp and Ln resolve to the single combined
        natural_log_exp_and_others set (true set indices are preserved; only
        the per-set function membership seen by the chooser shrinks)."""
        orig = bacc.get_activation_tables
        exp_t = mybir.ActivationFunctionType.Exp
        ln_t = mybir.ActivationFunctionType.Ln

        def patched(arch):
            tables = dict(orig(arch))
            if "natural_log_exp_and_others" not in tables:
                return tables
            out = {}
            for name, funcs in tables.items():
                if name != "natural_log_exp_and_others":
                    funcs = funcs - {exp_t, ln_t}
                out[name] = funcs
            return out

        bacc.get_activation_tables = patched
        try:
            yield
        finally:
            bacc.get_activation_tables = orig

    nc = bacc.Bacc()
    RP = nc.declare_dram_parameter("rp", [128, RP_W], bf16, isOutput=False)
    CA = nc.declare_dram_parameter("ca", [128, CA_W], f32, isOutput=False)
    UV = nc.declare_dram_parameter("uv", [20, 2 * L], bf16, isOutput=False)
    MK = nc.declare_dram_parameter("mk", [128, MK_W], bf16, isOutput=False)
    OUT = nc.declare_dram_parameter("out", [1, NB + 1], f32, isOutput=True)

    with ExitStack() as ctx:
        ctx.enter_context(nc.allow_low_precision(
            "bf16 block-sum outputs; 0.4% relative on positive sums is far "
            "inside the 2e-2 gate"))
        rp = ctx.enter_context(nc.sbuf_tensor([128, RP_W], bf16))
        ca = ctx.enter_context(nc.sbuf_tensor([128, CA_W], f32))
        uv = ctx.enter_context(nc.sbuf_tensor([20, 2 * L], bf16))
        mk = ctx.enter_context(nc.sbuf_tensor([128, MK_W], bf16))
        x3 = ctx.enter_context(nc.sbuf_tensor([128, L], bf16))
        hexp = ctx.enter_context(nc.sbuf_tensor([100, L], bf16))
        expb = ctx.enter_context(nc.sbuf_tensor([128, L], bf16))
        hsb = ctx.enter_context(nc.sbuf_tensor([100, NB], bf16))
        tmp = ctx.enter_context(nc.sbuf_tensor([100, NB], f32))
        gb = ctx.enter_context(nc.sbuf_tensor([128, NB], bf16))
        exc = ctx.enter_context(nc.sbuf_tensor([128, D * NB], f32))
        pdiag = ctx.enter_context(nc.sbuf_tensor([128, NB], bf16))
        lamA = ctx.enter_context(nc.sbuf_tensor([128, NB], f32))
        lamB = ctx.enter_context(nc.sbuf_tensor([128, NB], f32))
        acc = ctx.enter_context(nc.sbuf_tensor([128, NB + 1], f32))
        osb = ctx.enter_context(nc.sbuf_tensor([1, NB + 1], f32))
        warmt = ctx.enter_context(nc.sbuf_tensor([128, 512], bf16))
        psA = ctx.enter_context(nc.psum_tensor([128, 2048], f32))
        psB = ctx.enter_context(nc.psum_tensor([128, 2048], f32))
        warm_sem = ctx.enter_context(nc.semaphore("warm_sem"))
        rp_sem = ctx.enter_context(nc.semaphore("rp_sem"))
        ca_sem = ctx.enter_context(nc.semaphore("ca_sem"))
        uv_sem = ctx.enter_context(nc.semaphore("uv_sem"))
        mk_sem = ctx.enter_context(nc.semaphore("mk_sem"))
        pe_sem = ctx.enter_context(nc.semaphore("pe_sem"))
        act_sem = ctx.enter_context(nc.semaphore("act_sem"))
        dve_sem = ctx.enter_context(nc.semaphore("dve_sem"))
        gp_sem = ctx.enter_context(nc.semaphore("gp_sem"))
        out_sem = ctx.enter_context(nc.semaphore("out_sem"))
        block = ctx.enter_context(nc.Block(no_gpsimd_drain=True))
        rp, ca, uv, mk, x3, hexp, expb = (
            rp[:], ca[:], uv[:], mk[:], x3[:], hexp[:], expb[:])
        warmt = warmt[:]
        hsb, tmp, gb, exc, pdiag, lamA, lamB, acc, osb = (
            hsb[:], tmp[:], gb[:], exc[:], pdiag[:], lamA[:], lamB[:],
            acc[:], osb[:])
        psA, psB = psA[:], psB[:]

        mug = ca[:, MUG_OFF:MUG_OFF + NB]
        argc = ca[:, ARGC_OFF:ARGC_OFF + D * NB]
        onescol = ca[:, ONES_OFF:ONES_OFF + 1]
        delta = ca[0:100, DELTA_OFF:DELTA_OFF + NB]

        @block.sync
        def _(sync):
            sync.dma_start(out=rp, in_=RP[:]).then_inc(rp_sem, 16)
            sync.dma_start(out=ca, in_=CA[:]).then_inc(ca_sem, 16)
            sync.wait_ge(dve_sem, 11)
            sync.dma_start(out=OUT[:], in_=osb, single_packet=True).then_inc(out_sem, 16)

        @block.gpsimd
        def _(gp):
            gp.dma_start(out=uv, in_=UV[:]).then_inc(uv_sem, 16)
            gp.dma_start(out=mk, in_=MK[:]).then_inc(mk_sem, 16)
            nc.gpsimd.memset(tmp[:, 0:1], 0.0).then_inc(gp_sem, 1)
            # zero gb rows 96:128 (base must be 32-aligned); the scan later
            # overwrites rows 96:100 with real G values
            nc.gpsimd.memset(gb[96:128, :], 0.0).then_inc(gp_sem, 1)

        @block.tensor
        def _(pe):
            # HAM warm-up: big dummy matmuls while input DMAs land so the
            # real matmuls run at 2.4 GHz instead of the cold 1.2 GHz
            pe.wait_ge(warm_sem, 1)
            for _w in range(4):
                nc.tensor.matmul(
                    psB[:, 1024:1536], warmt[:, 0:128], warmt,
                    start=True, stop=True,
                )
            pe.wait_ge(rp_sem, 16)
            # H-args -> psA; four concurrent row-group matmuls
            for c in range(4):
                nc.tensor.matmul(
                    psA[:, 512 * c:512 * (c + 1)],
                    rp[32 * c:32 * c + 16, SH_OFF:SH_OFF + 128],
                    rp[32 * c:32 * c + 16, 0:512],
                    start=True, stop=True, tile_position=(32 * c, 0),
                ).then_inc(pe_sem, 1)                      # pe 1-4
            # X3-args -> psB
            for c in range(4):
                nc.tensor.matmul(
                    psB[:, 512 * c:512 * (c + 1)],
                    rp[32 * c:32 * c + 16, SX_OFF:SX_OFF + 128],
                    rp[32 * c:32 * c + 16, 0:512],
                    start=True, stop=True, tile_position=(32 * c, 0),
                ).then_inc(pe_sem, 1)                      # pe 5-8
            # band: -2048 strict-lower-triangle mask base, then W accumulates
            pe.wait_ge(mk_sem, 16)
            pe.wait_ge(uv_sem, 16)
            pe.wait_ge(act_sem, 1)   # psA half 1 free (H-exp chunk 1 read)
            for c in range(2):
                nc.tensor.matmul(
                    psA[:, 512 * c:512 * (c + 1)],
                    mk[:, 0:128],
                    mk[:, 128:640],
                    start=True, stop=False, skip_group_check=True,
                ).then_inc(pe_sem, 1)                      # pe 9-10
            pe.wait_ge(act_sem, 2)   # psA half 2 free
            for c in range(2, 4):
                nc.tensor.matmul(
                    psA[:, 512 * c:512 * (c + 1)],
                    mk[:, 0:128],
                    mk[:, 128:640],
                    start=True, stop=False, skip_group_check=True,
                ).then_inc(pe_sem, 1)                      # pe 11-12
            for k in range(NB):
                nc.tensor.matmul(
                    psA[:, 128 * k:128 * (k + 1)],
                    uv[:, 128 * k:128 * (k + 1)],
                    uv[:, L + 128 * k:L + 128 * (k + 1)],
                    start=False, stop=(k % 4 == 3), skip_group_check=True,
                ).then_inc(pe_sem, 1)                      # pe 13-28
            # P[p,k] = sum_{(a,d)} X3[(a,d), 128k+p] * G[(a,d), k]
            pe.wait_ge(act_sem, 3)   # psB free (X3-exp read), X3 ready
            pe.wait_ge(dve_sem, 4)   # G scan done
            pe.wait_ge(gp_sem, 2)    # gb rows 96:128 zeroed
            # open the P accumulation group with the mug base: psB[:,0:16] = I @ mug
            nc.tensor.matmul(
                psB[:, 0:NB],
                mk[:, 640:768],
                mk[:, 768:768 + NB],
                start=True, stop=False, skip_group_check=True,
            ).then_inc(pe_sem, 1)                          # pe 29
            for k in range(NB):
                nc.tensor.matmul(
                    psB[:, k:k + 1],
                    x3[:, 128 * k:128 * (k + 1)],
                    gb[:, k:k + 1],
                    start=False, stop=(k == NB - 1), skip_group_check=True,
                ).then_inc(pe_sem, 1)                      # pe 30-45
            # partition-sum of acc via ones-matmul -> (1, 17)
            pe.wait_ge(act_sem, 9)
            pe.wait_ge(dve_sem, 10)
            nc.tensor.matmul(
                psB[0:1, 512:512 + NB + 1],
                onescol,
                acc,
                start=True, stop=True,
            ).then_inc(pe_sem, 1)                          # pe 46

        @block.scalar
        def _(act):
            act.wait_ge(pe_sem, 2)
            nc.scalar.activation(out=hexp[:, 0:1024], in_=psA[0:100, 0:1024],
                                 func=Act.Exp).then_inc(act_sem, 1)  # act 1
            act.wait_ge(pe_sem, 4)
            nc.scalar.activation(out=hexp[:, 1024:2048], in_=psA[0:100, 1024:2048],
                                 func=Act.Exp).then_inc(act_sem, 1)  # act 2
            act.wait_ge(pe_sem, 8)
            nc.scalar.activation(out=x3, in_=psB, func=Act.Exp
                                 ).then_inc(act_sem, 1)    # act 3
            for c in range(4):
                act.wait_ge(pe_sem, 16 + 4 * c)  # band quarter c masked+summed
                nc.scalar.activation(
                    out=expb[:, 512 * c:512 * (c + 1)],
                    in_=psA[:, 512 * c:512 * (c + 1)],
                    func=Act.Exp).then_inc(act_sem, 1)     # act 4-7
            act.wait_ge(ca_sem, 16)
            nc.scalar.activation(out=exc, in_=argc, func=Act.Exp
                                 ).then_inc(act_sem, 1)    # act 8
            act.wait_ge(dve_sem, 9)
            nc.scalar.activation(out=acc[:, 0:NB], in_=lamB, func=Act.Ln
                                 ).then_inc(act_sem, 1)    # act 9

        @block.vector
        def _(dve):
            nc.vector.memset(warmt, 0.0).then_inc(warm_sem, 1)
            # H block sums (100, 16), two halves pipelined with H-exp
            dve.wait_ge(act_sem, 1)
            nc.vector.tensor_reduce(
                out=hsb[:, 0:8],
                in_=hexp[:, 0:1024].rearrange("p (k x) -> p k x", k=8),
                axis=mybir.AxisListType.X, op=Alu.add,
            ).then_inc(dve_sem, 1)                         # dve 1
            dve.wait_ge(act_sem, 2)
            nc.vector.tensor_reduce(
                out=hsb[:, 8:16],
                in_=hexp[:, 1024:2048].rearrange("p (k x) -> p k x", k=8),
                axis=mybir.AxisListType.X, op=Alu.add,
            ).then_inc(dve_sem, 1)                         # dve 2
            # tmp[:,k] = Delta[:,k] * H[:,k-1]
            dve.wait_ge(ca_sem, 16)
            dve.wait_ge(gp_sem, 2)   # tmp col0 + gb tail memsets done
            dve.wait_ge(dve_sem, 2)  # same-engine W->R on hsb
            nc.vector.tensor_tensor(
                out=tmp[:, 1:NB], in0=delta[:, 1:NB], in1=hsb[0:100, 0:NB - 1],
                op=Alu.mult,
            ).then_inc(dve_sem, 1)                         # dve 3
            # G[:,k] = Delta[:,k]*G[:,k-1] + tmp[:,k]   (bf16 out)
            dve.wait_ge(dve_sem, 3)  # same-engine W->R on tmp
            nc.vector.tensor_tensor_scan(
                out=gb[0:100, :], data0=delta, data1=tmp[0:100, :],
                initial=0.0, op0=Alu.mult, op1=Alu.add,
            ).then_inc(dve_sem, 1)                         # dve 4
            # band row sums, four quarters pipelined with band-exp
            for c in range(4):
                dve.wait_ge(act_sem, 4 + c)
                nc.vector.tensor_reduce(
                    out=pdiag[:, 4 * c:4 * (c + 1)],
                    in_=expb[:, 512 * c:512 * (c + 1)].rearrange(
                        "p (k x) -> p k x", k=4),
                    axis=mybir.AxisListType.X, op=Alu.add,
                ).then_inc(dve_sem, 1)                     # dve 5-8
            # lam = (P + mug) + pdiag
            dve.wait_ge(pe_sem, 45)
            dve.wait_ge(dve_sem, 8)  # same-engine W->R on pdiag
            nc.vector.tensor_tensor(
                out=lamB, in0=psB[:, 0:NB], in1=pdiag, op=Alu.add,
            ).then_inc(dve_sem, 1)                         # dve 9
            # compensator row sums (parallel with Ln)
            dve.wait_ge(act_sem, 8)
            nc.vector.tensor_reduce(
                out=acc[:, NB:NB + 1], in_=exc,
                axis=mybir.AxisListType.X, op=Alu.add,
            ).then_inc(dve_sem, 1)                         # dve 10
            # copy (1,17) result out of psum
            dve.wait_ge(pe_sem, 46)
            nc.vector.tensor_scalar_add(
                osb, psB[0:1, 512:512 + NB + 1], 0.0,
            ).then_inc(dve_sem, 1)                         # dve 11

    with _one_act_table():
        nc.finalize()
    return nc


def _softplus(x):
    return np.logaddexp(0.0, np.asarray(x, np.float64))


def _bf(x):
    import ml_dtypes
    return np.asarray(x, np.float32).astype(ml_dtypes.bfloat16)


def _host_prep(time_points, T, mu_raw, log_alpha, log_beta, event_types):
    """Per-core input tiles + additive host constants."""
    import ml_dtypes
    bf16 = ml_dtypes.bfloat16

    mu = _softplus(mu_raw).astype(np.float32)
    alpha = _softplus(log_alpha).astype(np.float32)
    beta = _softplus(log_beta).astype(np.float32)
    lnab = np.log(alpha.astype(np.float64) * beta.astype(np.float64)).astype(np.float32)
    lna = np.log(alpha.astype(np.float64)).astype(np.float32)
    colsumA = alpha.sum(0, dtype=np.float64)
    mu_sum = mu.sum(dtype=np.float64)

    b_flat = beta.reshape(-1).astype(np.float64)                  # b[a,d]
    A_flat = (alpha.astype(np.float64) * beta.astype(np.float64)).reshape(-1)
    lnA_flat = np.log(A_flat)
    bh = _bf(b_flat); bl = _bf(b_flat - bh.astype(np.float64))
    lAh = _bf(lnA_flat); lAl = _bf(lnA_flat - lAh.astype(np.float64))

    # stationaries (16, 128) f32 (bf16-representable values by construction)
    sel_d = np.zeros((D, P2), np.float32)
    sel_a = np.zeros((D, P2), np.float32)
    for a in range(D):
        for d in range(D):
            sel_d[d, D * a + d] = MASKC
            sel_a[a, D * a + d] = MASKC
    SX = np.zeros((16, 128), np.float32)
    SX[0, :] = -MASKC
    SX[1:11, :P2] = sel_a
    SX[11, :P2] = lAh.astype(np.float32)
    SX[12, :P2] = lAl.astype(np.float32)
    SX[13, :P2] = -bh.astype(np.float32)
    SX[14, :P2] = -bl.astype(np.float32)
    SX[15, :P2] = -bh.astype(np.float32)
    SH = np.zeros((16, 128), np.float32)
    SH[0, :] = -MASKC
    SH[1:11, :P2] = sel_d
    SH[13, :P2] = bh.astype(np.float32)
    SH[14, :P2] = bl.astype(np.float32)
    SH[15, :P2] = bh.astype(np.float32)

    # mk: lower-tri ones (m >= c) + 4 copies of -2048*I
    lt = (np.arange(128)[:, None] >= np.arange(128)[None, :]).astype(np.float32)
    negi = np.zeros((128, 512), np.float32)
    for r in range(4):
        negi[:, 128 * r:128 * (r + 1)] = -MASKC * np.eye(128, dtype=np.float32)
    mk_const = np.concatenate(
        [lt, negi, np.eye(128, dtype=np.float32)], axis=1)        # (128, 768)

    in_maps, consts = [], []
    for bb in range(B):
        t = np.asarray(time_points[bb], np.float32)
        e = np.asarray(event_types[bb], np.int64)
        Tb = np.float64(T[bb])

        c = t.reshape(NB, 128).mean(axis=1).astype(np.float32)
        tc = (t - np.repeat(c, 128)).astype(np.float32)
        tch = _bf(tc)
        tcl = _bf(tc - tch.astype(np.float32))
        E1 = np.zeros((L, D), np.float32)
        E1[np.arange(L), e] = 1.0

        one = np.ones(L, np.float32)
        rhs = np.stack(
            [one] + [E1[:, d] for d in range(D)]
            + [one, one, tch.astype(np.float32), tch.astype(np.float32),
               tcl.astype(np.float32)], axis=0)             # (16, L)
        rp = np.zeros((128, RP_W), np.float32)
        for cc in range(4):
            rp[32 * cc:32 * cc + 16, 0:512] = rhs[:, 512 * cc:512 * (cc + 1)]
            rp[32 * cc:32 * cc + 16, SX_OFF:SX_OFF + 128] = SX
            rp[32 * cc:32 * cc + 16, SH_OFF:SH_OFF + 128] = SH

        # Delta for the scan; col 0 = 0 resets the recursion
        Delta = np.zeros((100, NB), np.float32)
        dcs = np.diff(c.astype(np.float64))
        for k in range(1, NB):
            Delta[:, k] = np.exp(-b_flat * dcs[k - 1]).astype(np.float32)

        mug = mu[e].reshape(NB, 128).T.copy()               # (128, NB)
        dt = (np.float32(Tb) - t).astype(np.float32)
        argc = (lna[:, e] - beta[:, e] * dt[None, :]).astype(np.float32)
        argc = argc.reshape(D, NB, 128).transpose(2, 0, 1).reshape(128, D * NB)

        ca = np.zeros((128, CA_W), np.float32)
        ca[:, MUG_OFF:MUG_OFF + NB] = mug
        ca[:, ARGC_OFF:ARGC_OFF + D * NB] = argc
        ca[:, ONES_OFF] = 1.0
        ca[0:100, DELTA_OFF:DELTA_OFF + NB] = Delta

        # band stacks: W = U @ V^T per 128-block, times recentered at c_k
        U = np.empty((L, 2 * D), np.float32)
        U[:, :D] = lnab[e, :] - beta[e, :] * tc[:, None]
        U[:, D:] = beta[e, :]
        V = np.concatenate([E1, E1 * tc[:, None]], axis=1)
        uvt = np.concatenate([_bf(U).T, _bf(V).T], axis=1)  # (20, 2L)

        mkt = np.concatenate(
            [mk_const, mug.astype(np.float32)], axis=1).astype(bf16)
        const = -Tb * mu_sum - colsumA[e].sum()
        in_maps.append({
            "rp": np.ascontiguousarray(rp.astype(bf16)),
            "ca": np.ascontiguousarray(ca),
            "uv": np.ascontiguousarray(uvt.astype(bf16)),
            "mk": np.ascontiguousarray(mkt),
        })
        consts.append(const)
    return in_maps, consts


def kernel(**inputs):
    from concourse.bass_utils import run_bass_kernel_spmd

    if "nc" not in _CACHE:
        _CACHE["nc"] = _build_nc()
    nc = _CACHE["nc"]

    in_maps, consts = _host_prep(**inputs)
    res = run_bass_kernel_spmd(nc, in_maps, list(range(NCORES)))
    out = np.empty(B, np.float32)
    for bb in range(B):
        row = res.results[bb]["out"].reshape(NB + 1).astype(np.float64)
        out[bb] = np.float32(row.sum() + consts[bb])
    return out


# revision 21
# speedup vs baseline: 1.0436x; 1.0049x over previous
"""Exp-kernel multivariate Hawkes process log-likelihood on 8 Trainium2 cores.

Data-parallel: one sequence (L=2048) per core. The O(L^2) pairwise sum is
split into (a) a within-block band (16 blocks of 128) computed densely via
a K=20 bf16 one-hot matmul + exp, and (b) the strictly-older "prefix" part
computed in O(L*D^2) via per-type-pair sufficient statistics (Ogata-style):

  prefix_i = sum_{(a,d)} X3[(a,d), i] * G[(a,d), k(i)]
  X3[(a,d), i] = [e_i = a] * alpha*beta[a,d] * exp(-b[a,d](t_i - c_k))
  G[(a,d), k]  = sum_{j < 128k, e_j = d} exp(-b[a,d](c_k - t_j))

H (per-block partial sums) and G (cross-block decayed scan) are carried on
the vector engine with a single tensor_tensor_scan; the per-event combine is
16 tiny PE matmuls contracting the 100 type-pairs. Arguments for the two big
exp() calls are built on the tensor engine from one shared (16, 2048) bf16
rhs with compensated hi/lo rows; masking of wrong-type entries uses -2048
offsets that underflow exp to exactly 0. The strict-triangle band mask is
accumulated in PSUM by 4 extra matmuls (lower-tri ones stationary times
-2048*I moving) so the scalar engine exps masked PSUM directly. Big exp
outputs are bf16, which doubles vector-engine reduction throughput. Total
scalar-engine exp work is ~3x2048 elements/lane (vs ~17400 for the dense
O(L^2) triangle).

Final ln(lam) runs on-device (exp+ln share one ACT table set via a scoped
patch of the table catalog passed to the insertion pass), and the result is
collapsed to a (1, 17) row with a ones-matmul so the output DMA is a single
packet (a (128, 1) output costs ~7us in straggling per-engine completions).

Engine programs and semaphores are hand-written (raw bacc, no Tile context).
"""
import numpy as np

B, L, D = 8, 2048, 10
NB = L // 128            # 16 blocks of 128 events
P2 = D * D               # 100 type pairs (a, d)
MASKC = 2048.0           # exp(-2048 + small) == 0, and 2048 >> |args| keeps
                         # fp32 accumulation exact to ~2^-12
NCORES = 8

# rp (shared rhs + stationaries), bf16, (128, 768). Strip c (partitions
# 32c:32c+16) carries rhs chunk c plus its own stationary copies so the four
# arg matmuls run concurrently in separate PE row groups:
#   cols 0:512     rhs rows over j-chunk c: [one, E1 d=0..9, one, one, tch,
#                  tch, tcl]
#   cols 512:640   SX stationary (X3-args), col 100:128 -> -MASKC row0 only
#   cols 640:768   SH stationary (H-args)
RHS_W = 512
SX_OFF = 512
SH_OFF = 640
RP_W = 768

# cA f32 (128, 193): [mug 0:16 | argc 16:176 | ones 176 | Delta 177:193]
MUG_OFF = 0
ARGC_OFF = 16
ONES_OFF = 176
DELTA_OFF = 177
CA_W = 193

# mk bf16 (128, 784): [LT ones (m>=c) 0:128 | 4x -2048*I 128:640 |
#                      I128 640:768 | mug 768:784]
MK_W = 784

_CACHE = {}


def _build_nc():
    import concourse.bacc as bacc
    from concourse import mybir
    from contextlib import ExitStack, contextmanager

    f32 = mybir.dt.float32
    bf16 = mybir.dt.bfloat16
    Alu = mybir.AluOpType
    Act = mybir.ActivationFunctionType

    @contextmanager
    def _one_act_table():
        """Scope-patch the activation-table catalog fed to bass's table-load
        insertion pass so Exp and Ln resolve to the single combined
        natural_log_exp_and_others set (true set indices are preserved; only
        the per-set function membership seen by the chooser shrinks)."""
        orig = bacc.get_activation_tables
        exp_t = mybir.ActivationFunctionType.Exp
        ln_t = mybir.ActivationFunctionType.Ln

        def patched(arch):
            tables = dict(orig(arch))
            if "natural_log_exp_and_others" not in tables:
                return tables
            out = {}
            for name, funcs in tables.items():
                if name != "natural_log_exp_and_others":
                    funcs = funcs - {exp_t, ln_t}
                out[name] = funcs
            return out

        bacc.get_activation_tables = patched
        try:
            yield
        finally:
            bacc.get_activation_tables = orig

    nc = bacc.Bacc()
    RP = nc.declare_dram_parameter("rp", [128, RP_W], bf16, isOutput=False)
    CA = nc.declare_dram_parameter("ca", [128, CA_W], f32, isOutput=False)
    UV = nc.declare_dram_parameter("uv", [20, 2 * L], bf16, isOutput=False)
    MK = nc.declare_dram_parameter("mk", [128, MK_W], bf16, isOutput=False)
    OUT = nc.declare_dram_parameter("out", [1, NB + 1], f32, isOutput=True)

    with ExitStack() as ctx:
        ctx.enter_context(nc.allow_low_precision(
            "bf16 block-sum outputs; 0.4% relative on positive sums is far "
            "inside the 2e-2 gate"))
        rp = ctx.enter_context(nc.sbuf_tensor([128, RP_W], bf16))
        ca = ctx.enter_context(nc.sbuf_tensor([128, CA_W], f32))
        uv = ctx.enter_context(nc.sbuf_tensor([20, 2 * L], bf16))
        mk = ctx.enter_context(nc.sbuf_tensor([128, MK_W], bf16))
        x3 = ctx.enter_context(nc.sbuf_tensor([128, L], bf16))
        hexp = ctx.enter_context(nc.sbuf_tensor([100, L], bf16))
        expb = ctx.enter_context(nc.sbuf_tensor([128, L], bf16))
        hsb = ctx.enter_context(nc.sbuf_tensor([100, NB], bf16))
        tmp = ctx.enter_context(nc.sbuf_tensor([100, NB], f32))
        gb = ctx.enter_context(nc.sbuf_tensor([128, NB], bf16))
        exc = ctx.enter_context(nc.sbuf_tensor([128, D * NB], f32))
        pdiag = ctx.enter_context(nc.sbuf_tensor([128, NB], bf16))
        lamA = ctx.enter_context(nc.sbuf_tensor([128, NB], f32))
        lamB = ctx.enter_context(nc.sbuf_tensor([128, NB], f32))
        acc = ctx.enter_context(nc.sbuf_tensor([128, NB + 1], f32))
        osb = ctx.enter_context(nc.sbuf_tensor([1, NB + 1], f32))
        warmt = ctx.enter_context(nc.sbuf_tensor([128, 512], bf16))
        psA = ctx.enter_context(nc.psum_tensor([128, 2048], f32))
        psB = ctx.enter_context(nc.psum_tensor([128, 2048], f32))
        warm_sem = ctx.enter_context(nc.semaphore("warm_sem"))
        rp_sem = ctx.enter_context(nc.semaphore("rp_sem"))
        ca_sem = ctx.enter_context(nc.semaphore("ca_sem"))
        uv_sem = ctx.enter_context(nc.semaphore("uv_sem"))
        mk_sem = ctx.enter_context(nc.semaphore("mk_sem"))
        pe_sem = ctx.enter_context(nc.semaphore("pe_sem"))
        act_sem = ctx.enter_context(nc.semaphore("act_sem"))
        dve_sem = ctx.enter_context(nc.semaphore("dve_sem"))
        gp_sem = ctx.enter_context(nc.semaphore("gp_sem"))
        out_sem = ctx.enter_context(nc.semaphore("out_sem"))
        block = ctx.enter_context(nc.Block(no_gpsimd_drain=True))
        rp, ca, uv, mk, x3, hexp, expb = (
            rp[:], ca[:], uv[:], mk[:], x3[:], hexp[:], expb[:])
        warmt = warmt[:]
        hsb, tmp, gb, exc, pdiag, lamA, lamB, acc, osb = (
            hsb[:], tmp[:], gb[:], exc[:], pdiag[:], lamA[:], lamB[:],
            acc[:], osb[:])
        psA, psB = psA[:], psB[:]

        mug = ca[:, MUG_OFF:MUG_OFF + NB]
        argc = ca[:, ARGC_OFF:ARGC_OFF + D * NB]
        onescol = ca[:, ONES_OFF:ONES_OFF + 1]
        delta = ca[0:100, DELTA_OFF:DELTA_OFF + NB]

        @block.sync
        def _(sync):
            sync.dma_start(out=rp, in_=RP[:]).then_inc(rp_sem, 16)
            sync.dma_start(out=ca, in_=CA[:]).then_inc(ca_sem, 16)
            sync.wait_ge(dve_sem, 10)
            sync.dma_start(out=OUT[:], in_=osb, single_packet=True).then_inc(out_sem, 16)

        @block.gpsimd
        def _(gp):
            gp.dma_start(out=uv, in_=UV[:]).then_inc(uv_sem, 16)
            gp.dma_start(out=mk, in_=MK[:]).then_inc(mk_sem, 16)
            nc.gpsimd.memset(tmp[:, 0:1], 0.0).then_inc(gp_sem, 1)
            # zero gb rows 96:128 (base must be 32-aligned); the scan later
            # overwrites rows 96:100 with real G values
            nc.gpsimd.memset(gb[96:128, :], 0.0).then_inc(gp_sem, 1)

        @block.tensor
        def _(pe):
            # HAM warm-up: big dummy matmuls while input DMAs land so the
            # real matmuls run at 2.4 GHz instead of the cold 1.2 GHz
            pe.wait_ge(warm_sem, 1)
            for _w in range(4):
                nc.tensor.matmul(
                    psB[:, 1024:1536], warmt[:, 0:128], warmt,
                    start=True, stop=True,
                )
            pe.wait_ge(rp_sem, 16)
            # H-args -> psA; four concurrent row-group matmuls
            for c in range(4):
                nc.tensor.matmul(
                    psA[:, 512 * c:512 * (c + 1)],
                    rp[32 * c:32 * c + 16, SH_OFF:SH_OFF + 128],
                    rp[32 * c:32 * c + 16, 0:512],
                    start=True, stop=True, tile_position=(32 * c, 0),
                ).then_inc(pe_sem, 1)                      # pe 1-4
            # X3-args -> psB
            for c in range(4):
                nc.tensor.matmul(
                    psB[:, 512 * c:512 * (c + 1)],
                    rp[32 * c:32 * c + 16, SX_OFF:SX_OFF + 128],
                    rp[32 * c:32 * c + 16, 0:512],
                    start=True, stop=True, tile_position=(32 * c, 0),
                ).then_inc(pe_sem, 1)                      # pe 5-8
            # band: -2048 strict-lower-triangle mask base, then W accumulates
            pe.wait_ge(mk_sem, 16)
            pe.wait_ge(uv_sem, 16)
            pe.wait_ge(act_sem, 1)   # psA half 1 free (H-exp chunk 1 read)
            for c in range(2):
                nc.tensor.matmul(
                    psA[:, 512 * c:512 * (c + 1)],
                    mk[:, 0:128],
                    mk[:, 128:640],
                    start=True, stop=False, skip_group_check=True,
                ).then_inc(pe_sem, 1)                      # pe 9-10
            pe.wait_ge(act_sem, 2)   # psA half 2 free
            for c in range(2, 4):
                nc.tensor.matmul(
                    psA[:, 512 * c:512 * (c + 1)],
                    mk[:, 0:128],
                    mk[:, 128:640],
                    start=True, stop=False, skip_group_check=True,
                ).then_inc(pe_sem, 1)                      # pe 11-12
            for k in range(NB):
                nc.tensor.matmul(
                    psA[:, 128 * k:128 * (k + 1)],
                    uv[:, 128 * k:128 * (k + 1)],
                    uv[:, L + 128 * k:L + 128 * (k + 1)],
                    start=False, stop=(k % 4 == 3), skip_group_check=True,
                ).then_inc(pe_sem, 1)                      # pe 13-28
            # P[p,k] = sum_{(a,d)} X3[(a,d), 128k+p] * G[(a,d), k]
            pe.wait_ge(act_sem, 3)   # psB free (X3-exp read), X3 ready
            pe.wait_ge(dve_sem, 4)   # G scan done
            pe.wait_ge(gp_sem, 2)    # gb rows 96:128 zeroed
            # open the P accumulation group with the mug base: psB[:,0:16] = I @ mug
            nc.tensor.matmul(
                psB[:, 0:NB],
                mk[:, 640:768],
                mk[:, 768:768 + NB],
                start=True, stop=False, skip_group_check=True,
            ).then_inc(pe_sem, 1)                          # pe 29
            for k in range(NB):
                nc.tensor.matmul(
                    psB[:, k:k + 1],
                    x3[:, 128 * k:128 * (k + 1)],
                    gb[:, k:k + 1],
                    start=False, stop=(k == NB - 1), skip_group_check=True,
                ).then_inc(pe_sem, 1)                      # pe 30-45
            # accumulate pdiag into the P columns: psB[:,0:16] += I @ pdiag
            pe.wait_ge(dve_sem, 8)   # band row sums done
            nc.tensor.matmul(
                psB[:, 0:NB],
                mk[:, 640:768],
                pdiag,
                start=False, stop=True, skip_group_check=True,
            ).then_inc(pe_sem, 1)                          # pe 46
            # partition-sum of acc via ones-matmul -> (1, 17)
            pe.wait_ge(act_sem, 9)
            pe.wait_ge(dve_sem, 9)
            nc.tensor.matmul(
                psB[0:1, 512:512 + NB + 1],
                onescol,
                acc,
                start=True, stop=True,
            ).then_inc(pe_sem, 1)                          # pe 47

        @block.scalar
        def _(act):
            act.wait_ge(pe_sem, 2)
            nc.scalar.activation(out=hexp[:, 0:1024], in_=psA[0:100, 0:1024],
                                 func=Act.Exp).then_inc(act_sem, 1)  # act 1
            act.wait_ge(pe_sem, 4)
            nc.scalar.activation(out=hexp[:, 1024:2048], in_=psA[0:100, 1024:2048],
                                 func=Act.Exp).then_inc(act_sem, 1)  # act 2
            act.wait_ge(pe_sem, 8)
            nc.scalar.activation(out=x3, in_=psB, func=Act.Exp
                                 ).then_inc(act_sem, 1)    # act 3
            for c in range(4):
                act.wait_ge(pe_sem, 16 + 4 * c)  # band quarter c masked+summed
                nc.scalar.activation(
                    out=expb[:, 512 * c:512 * (c + 1)],
                    in_=psA[:, 512 * c:512 * (c + 1)],
                    func=Act.Exp).then_inc(act_sem, 1)     # act 4-7
            act.wait_ge(ca_sem, 16)
            nc.scalar.activation(out=exc, in_=argc, func=Act.Exp
                                 ).then_inc(act_sem, 1)    # act 8
            act.wait_ge(pe_sem, 46)
            nc.scalar.activation(out=acc[:, 0:NB], in_=psB[:, 0:NB], func=Act.Ln
                                 ).then_inc(act_sem, 1)    # act 9

        @block.vector
        def _(dve):
            nc.vector.memset(warmt, 0.0).then_inc(warm_sem, 1)
            # H block sums (100, 16), two halves pipelined with H-exp
            dve.wait_ge(act_sem, 1)
            nc.vector.tensor_reduce(
                out=hsb[:, 0:8],
                in_=hexp[:, 0:1024].rearrange("p (k x) -> p k x", k=8),
                axis=mybir.AxisListType.X, op=Alu.add,
            ).then_inc(dve_sem, 1)                         # dve 1
            dve.wait_ge(act_sem, 2)
            nc.vector.tensor_reduce(
                out=hsb[:, 8:16],
                in_=hexp[:, 1024:2048].rearrange("p (k x) -> p k x", k=8),
                axis=mybir.AxisListType.X, op=Alu.add,
            ).then_inc(dve_sem, 1)                         # dve 2
            # tmp[:,k] = Delta[:,k] * H[:,k-1]
            dve.wait_ge(ca_sem, 16)
            dve.wait_ge(gp_sem, 2)   # tmp col0 + gb tail memsets done
            dve.wait_ge(dve_sem, 2)  # same-engine W->R on hsb
            nc.vector.tensor_tensor(
                out=tmp[:, 1:NB], in0=delta[:, 1:NB], in1=hsb[0:100, 0:NB - 1],
                op=Alu.mult,
            ).then_inc(dve_sem, 1)                         # dve 3
            # G[:,k] = Delta[:,k]*G[:,k-1] + tmp[:,k]   (bf16 out)
            dve.wait_ge(dve_sem, 3)  # same-engine W->R on tmp
            nc.vector.tensor_tensor_scan(
                out=gb[0:100, :], data0=delta, data1=tmp[0:100, :],
                initial=0.0, op0=Alu.mult, op1=Alu.add,
            ).then_inc(dve_sem, 1)                         # dve 4
            # band row sums, four quarters pipelined with band-exp
            for c in range(4):
                dve.wait_ge(act_sem, 4 + c)
                nc.vector.tensor_reduce(
                    out=pdiag[:, 4 * c:4 * (c + 1)],
                    in_=expb[:, 512 * c:512 * (c + 1)].rearrange(
                        "p (k x) -> p k x", k=4),
                    axis=mybir.AxisListType.X, op=Alu.add,
                ).then_inc(dve_sem, 1)                     # dve 5-8
            # compensator row sums (parallel with Ln)
            dve.wait_ge(act_sem, 8)
            nc.vector.tensor_reduce(
                out=acc[:, NB:NB + 1], in_=exc,
                axis=mybir.AxisListType.X, op=Alu.add,
            ).then_inc(dve_sem, 1)                         # dve 9
            # copy (1,17) result out of psum
            dve.wait_ge(pe_sem, 47)
            nc.vector.tensor_scalar_add(
                osb, psB[0:1, 512:512 + NB + 1], 0.0,
            ).then_inc(dve_sem, 1)                         # dve 10

    with _one_act_table():
        nc.finalize()
    return nc


def _softplus(x):
    return np.logaddexp(0.0, np.asarray(x, np.float64))


def _bf(x):
    import ml_dtypes
    return np.asarray(x, np.float32).astype(ml_dtypes.bfloat16)


def _host_prep(time_points, T, mu_raw, log_alpha, log_beta, event_types):
    """Per-core input tiles + additive host constants."""
    import ml_dtypes
    bf16 = ml_dtypes.bfloat16

    mu = _softplus(mu_raw).astype(np.float32)
    alpha = _softplus(log_alpha).astype(np.float32)
    beta = _softplus(log_beta).astype(np.float32)
    lnab = np.log(alpha.astype(np.float64) * beta.astype(np.float64)).astype(np.float32)
    lna = np.log(alpha.astype(np.float64)).astype(np.float32)
    colsumA = alpha.sum(0, dtype=np.float64)
    mu_sum = mu.sum(dtype=np.float64)

    b_flat = beta.reshape(-1).astype(np.float64)                  # b[a,d]
    A_flat = (alpha.astype(np.float64) * beta.astype(np.float64)).reshape(-1)
    lnA_flat = np.log(A_flat)
    bh = _bf(b_flat); bl = _bf(b_flat - bh.astype(np.float64))
    lAh = _bf(lnA_flat); lAl = _bf(lnA_flat - lAh.astype(np.float64))

    # stationaries (16, 128) f32 (bf16-representable values by construction)
    sel_d = np.zeros((D, P2), np.float32)
    sel_a = np.zeros((D, P2), np.float32)
    for a in range(D):
        for d in range(D):
            sel_d[d, D * a + d] = MASKC
            sel_a[a, D * a + d] = MASKC
    SX = np.zeros((16, 128), np.float32)
    SX[0, :] = -MASKC
    SX[1:11, :P2] = sel_a
    SX[11, :P2] = lAh.astype(np.float32)
    SX[12, :P2] = lAl.astype(np.float32)
    SX[13, :P2] = -bh.astype(np.float32)
    SX[14, :P2] = -bl.astype(np.float32)
    SX[15, :P2] = -bh.astype(np.float32)
    SH = np.zeros((16, 128), np.float32)
    SH[0, :] = -MASKC
    SH[1:11, :P2] = sel_d
    SH[13, :P2] = bh.astype(np.float32)
    SH[14, :P2] = bl.astype(np.float32)
    SH[15, :P2] = bh.astype(np.float32)

    # mk: lower-tri ones (m >= c) + 4 copies of -2048*I
    lt = (np.arange(128)[:, None] >= np.arange(128)[None, :]).astype(np.float32)
    negi = np.zeros((128, 512), np.float32)
    for r in range(4):
        negi[:, 128 * r:128 * (r + 1)] = -MASKC * np.eye(128, dtype=np.float32)
    mk_const = np.concatenate(
        [lt, negi, np.eye(128, dtype=np.float32)], axis=1)        # (128, 768)

    in_maps, consts = [], []
    for bb in range(B):
        t = np.asarray(time_points[bb], np.float32)
        e = np.asarray(event_types[bb], np.int64)
        Tb = np.float64(T[bb])

        c = t.reshape(NB, 128).mean(axis=1).astype(np.float32)
        tc = (t - np.repeat(c, 128)).astype(np.float32)
        tch = _bf(tc)
        tcl = _bf(tc - tch.astype(np.float32))
        E1 = np.zeros((L, D), np.float32)
        E1[np.arange(L), e] = 1.0

        one = np.ones(L, np.float32)
        rhs = np.stack(
            [one] + [E1[:, d] for d in range(D)]
            + [one, one, tch.astype(np.float32), tch.astype(np.float32),
               tcl.astype(np.float32)], axis=0)             # (16, L)
        rp = np.zeros((128, RP_W), np.float32)
        for cc in range(4):
            rp[32 * cc:32 * cc + 16, 0:512] = rhs[:, 512 * cc:512 * (cc + 1)]
            rp[32 * cc:32 * cc + 16, SX_OFF:SX_OFF + 128] = SX
            rp[32 * cc:32 * cc + 16, SH_OFF:SH_OFF + 128] = SH

        # Delta for the scan; col 0 = 0 resets the recursion
        Delta = np.zeros((100, NB), np.float32)
        dcs = np.diff(c.astype(np.float64))
        for k in range(1, NB):
            Delta[:, k] = np.exp(-b_flat * dcs[k - 1]).astype(np.float32)

        mug = mu[e].reshape(NB, 128).T.copy()               # (128, NB)
        dt = (np.float32(Tb) - t).astype(np.float32)
        argc = (lna[:, e] - beta[:, e] * dt[None, :]).astype(np.float32)
        argc = argc.reshape(D, NB, 128).transpose(2, 0, 1).reshape(128, D * NB)

        ca = np.zeros((128, CA_W), np.float32)
        ca[:, MUG_OFF:MUG_OFF + NB] = mug
        ca[:, ARGC_OFF:ARGC_OFF + D * NB] = argc
        ca[:, ONES_OFF] = 1.0
        ca[0:100, DELTA_OFF:DELTA_OFF + NB] = Delta

        # band stacks: W = U @ V^T per 128-block, times recentered at c_k
        U = np.empty((L, 2 * D), np.float32)
        U[:, :D] = lnab[e, :] - beta[e, :] * tc[:, None]
        U[:, D:] = beta[e, :]
        V = np.concatenate([E1, E1 * tc[:, None]], axis=1)
        uvt = np.concatenate([_bf(U).T, _bf(V).T], axis=1)  # (20, 2L)

        mkt = np.concatenate(
            [mk_const, mug.astype(np.float32)], axis=1).astype(bf16)
        const = -Tb * mu_sum - colsumA[e].sum()
        in_maps.append({
            "rp": np.ascontiguousarray(rp.astype(bf16)),
            "ca": np.ascontiguousarray(ca),
            "uv": np.ascontiguousarray(uvt.astype(bf16)),
            "mk": np.ascontiguousarray(mkt),
        })
        consts.append(const)
    return in_maps, consts


def kernel(**inputs):
    from concourse.bass_utils import run_bass_kernel_spmd

    if "nc" not in _CACHE:
        _CACHE["nc"] = _build_nc()
    nc = _CACHE["nc"]

    in_maps, consts = _host_prep(**inputs)
    res = run_bass_kernel_spmd(nc, in_maps, list(range(NCORES)))
    out = np.empty(B, np.float32)
    for bb in range(B):
        row = res.results[bb]["out"].reshape(NB + 1).astype(np.float64)
        out[bb] = np.float32(row.sum() + consts[bb])
    return out
